# revision 9
# baseline (speedup 1.0000x reference)
"""NemotronH Mamba2 mixer on 8 Trainium2 cores (Bass/Tile).

Sharding: tensor-parallel over heads/groups. Core c owns group c =
16 heads (= 1024 gate/x channels, 128 B + 128 C state channels, 16 dt).
in_proj rows and out_proj columns are sharded accordingly; out_proj is
row-parallel over the contraction, partials are combined on the host.

Per-core dataflow (all seq-major blocks of 512, chunks of 128):
  in_proj (fp32r matmul) -> depthwise conv (DVE taps) + SiLU
  -> Mamba2 chunked SSD (PE matmuls per chunk/head) -> gated group
  RMSNorm -> out_proj (fp32r matmul) -> partial [4096, 2048] output.
"""

import numpy as np

import concourse.bass as bass
import concourse.mybir as mybir
from concourse import bacc
from concourse.tile import TileContext
from concourse.bass_utils import run_bass_kernel_spmd

F32 = mybir.dt.float32
F32R = mybir.dt.float32r
AF = mybir.ActivationFunctionType
ALU = mybir.AluOpType

# Model dims
H_SIZE = 4096
NH = 128
HD = 64
SS = 128
KCONV = 4
NG = 8
CHUNK = 128
INTER = NH * HD                 # 8192
CONV_DIM = INTER + 2 * NG * SS  # 10240
PROJ = INTER + CONV_DIM + NH    # 18560
DT_MIN, DT_MAX = 0.001, 100.0
EPS = 1e-5
GROUP = INTER // NG             # 1024

# Sharding / tiling
N_CORES = 8
S = 2048
HL = NH // N_CORES              # 16 local heads
CLOC = HL * HD                  # 1024 local gate/x channels
NSB = 4                         # seq superblocks
SB = S // NSB                   # 512
NCPB = SB // CHUNK              # 4 chunks per superblock
NCH = S // CHUNK                # 16 chunks
NF = 19                         # in_proj f-tiles (2432 = 19*128, padded)
NK1 = H_SIZE // 128             # 32 k-tiles for in_proj
NK2 = CLOC // 128               # 8 k-tiles for out_proj
NM2 = H_SIZE // 128             # 32 m-tiles for out_proj
NEGM = -1e30

# log1p(u)/u on [0,1], Chebyshev-fit degree 12 (max fp32 err ~1.1e-7)
LOG1P_C = [0.9999999999815061, -0.4999999935552795, 0.33333295899388315,
           -0.2499913901062215, 0.19989602251462296, -0.1659083573590588,
           0.1392317246686566, -0.1130135727826319, 0.08261769871302305,
           -0.04960969557400616, 0.021956439674455992, -0.006180556818034449,
           0.0008159022224092772]

_CACHE = {}


def r32(ap):
    return ap.bitcast(F32R)


def bc_ap(src_ap, steps):
    """Stride-0 broadcast access pattern (DMA-only)."""
    return bass.AP(tensor=src_ap.tensor, offset=src_ap.offset, ap=steps)


def build_nc():
    nc = bacc.Bacc(None, target_bir_lowering=False)

    hidT = nc.declare_dram_parameter("hidT", [H_SIZE, S], F32, isOutput=False)
    w1t = nc.declare_dram_parameter("w1t", [NK1, NF, 128, 128], F32, isOutput=False)
    w2t = nc.declare_dram_parameter("w2t", [NK2, NM2, 128, 128], F32, isOutput=False)
    convw = nc.declare_dram_parameter("convw", [128, 10 * KCONV], F32, isOutput=False)
    convb = nc.declare_dram_parameter("convb", [128, 10], F32, isOutput=False)
    dtbias = nc.declare_dram_parameter("dtbias", [HL, 1], F32, isOutput=False)
    acol = nc.declare_dram_parameter("acol", [HL, 1], F32, isOutput=False)
    dbc = nc.declare_dram_parameter("dbc", [128, HL], F32, isOutput=False)
    normwbc = nc.declare_dram_parameter("normwbc", [128, CLOC], F32, isOutput=False)
    negmask = nc.declare_dram_parameter("negmask", [128, 128], F32, isOutput=False)
    ident = nc.declare_dram_parameter("ident", [128, 128], F32, isOutput=False)
    outp = nc.declare_dram_parameter("outp", [H_SIZE, S], F32, isOutput=True)

    with TileContext(nc) as tc:
        with tc.tile_pool(name="const", bufs=1) as cp, \
             tc.tile_pool(name="dram", bufs=1, space="DRAM") as dp:
            id_sb = cp.tile([128, 128], F32, tag="id")
            nm_sb = cp.tile([128, 128], F32, tag="nm")
            nw_sb = cp.tile([128, CLOC], F32, tag="nw")
            dbc_sb = cp.tile([128, HL], F32, tag="dbc")
            cw_sb = cp.tile([128, 10 * KCONV], F32, tag="cw")
            cb_sb = cp.tile([128, 10], F32, tag="cb")
            dtb_sb = cp.tile([HL, 1], F32, tag="dtb")
            a_sb = cp.tile([HL, 1], F32, tag="acol")
            ones16 = cp.tile([HL, CHUNK], F32, tag="ones16")
            st_sb = cp.tile([128, HL * HD], F32, tag="state")
            nc.sync.dma_start(out=id_sb[:], in_=ident[:])
            nc.sync.dma_start(out=nm_sb[:], in_=negmask[:])
            nc.sync.dma_start(out=nw_sb[:], in_=normwbc[:])
            nc.sync.dma_start(out=dbc_sb[:], in_=dbc[:])
            nc.sync.dma_start(out=cw_sb[:], in_=convw[:])
            nc.sync.dma_start(out=cb_sb[:], in_=convb[:])
            nc.sync.dma_start(out=dtb_sb[:], in_=dtbias[:])
            nc.sync.dma_start(out=a_sb[:], in_=acol[:])
            nc.vector.memset(ones16[:], 1.0)
            nc.vector.memset(st_sb[:], 0.0)

            normT = dp.tile([CLOC, S], F32, tag="normT")
            cs_dram = dp.tile([HL, S], F32, tag="cs_dram")

            _main_phase(nc, tc, hidT, w1t, id_sb, nm_sb, nw_sb, dbc_sb,
                        cw_sb, cb_sb, dtb_sb, a_sb, ones16, st_sb, normT,
                        cs_dram)
            _out_proj_phase(nc, tc, w2t, normT, outp)

    nc.compile()
    return nc


def _main_phase(nc, tc, hidT, w1t, id_sb, nm_sb, nw_sb, dbc_sb,
                cw_sb, cb_sb, dtb_sb, a_sb, ones16, st_sb, normT,
                cs_dram):
    with tc.tile_pool(name="hid", bufs=2) as hidp, \
         tc.tile_pool(name="w1", bufs=6) as w1p, \
         tc.tile_pool(name="proj", bufs=1) as projp, \
         tc.tile_pool(name="conv", bufs=1) as convp, \
         tc.tile_pool(name="dtl", bufs=1) as dtp, \
         tc.tile_pool(name="chunk", bufs=2) as chp, \
         tc.tile_pool(name="chunk1", bufs=1) as ch1p, \
         tc.tile_pool(name="heads", bufs=3) as hp, \
         tc.tile_pool(name="psA", bufs=1, space="PSUM") as psA, \
         tc.tile_pool(name="psS", bufs=1, space="PSUM") as psS, \
         tc.tile_pool(name="psT", bufs=1, space="PSUM") as psT, \
         tc.tile_pool(name="psY", bufs=2, space="PSUM") as psY:

        # convcat: 10 conv channel tiles (8 x, 1 B, 1 C), each 3 halo + 512
        ccat = convp.tile([128, 10 * (SB + 3)], F32, tag="ccat")
        for t in range(10):
            nc.vector.memset(ccat[:, t * (SB + 3):t * (SB + 3) + 3], 0.0)

        for sb in range(NSB):
            # ---------------- in_proj for this superblock ----------------
            halves = []
            for khalf in range(2):
                hid_h = hidp.tile([128, 16 * SB], F32R, tag="hid")
                for kk in range(16):
                    k = khalf * 16 + kk
                    nc.sync.dma_start(
                        out=hid_h[:, kk * SB:(kk + 1) * SB],
                        in_=r32(hidT[k * 128:(k + 1) * 128, sb * SB:(sb + 1) * SB]))
                halves.append(hid_h)

            gate_sb = projp.tile([128, 8 * SB], F32, tag="gate")
            dtraw = dtp.tile([HL, SB], F32, tag="dtraw")

            # halo copies must read previous superblock before overwrite
            if sb > 0:
                for t in range(10):
                    base = t * (SB + 3)
                    nc.vector.tensor_copy(
                        ccat[:, base:base + 3], ccat[:, base + SB:base + SB + 3])

            for f in range(NF):
                acc = psA.tile([128, SB], F32, tag="ipacc")
                for k in range(NK1):
                    w1 = w1p.tile([128, 128], F32R, tag="w1")
                    nc.sync.dma_start(out=w1[:], in_=r32(w1t[k, f]))
                    nc.tensor.matmul(
                        acc[:], w1[:],
                        halves[k // 16][:, (k % 16) * SB:(k % 16 + 1) * SB],
                        start=(k == 0), stop=(k == NK1 - 1))
                if f < 8:
                    nc.scalar.copy(gate_sb[:, f * SB:(f + 1) * SB], acc[:])
                elif f < 18:
                    t = f - 8
                    base = t * (SB + 3)
                    nc.scalar.copy(ccat[:, base + 3:base + 3 + SB], acc[:])
                else:
                    nc.scalar.copy(dtraw[:, :], acc[:HL, :])

            # ---------------- dt pipeline ----------------
            # softplus(z) = relu(z) + log1p(exp(-|z|)); log1p via poly
            # (no Softplus/Ln activation table on gen3)
            dtsp = dtp.tile([HL, SB], F32, tag="dtsp")
            dA = dtp.tile([HL, SB], F32, tag="dA")
            cs = dtp.tile([HL, SB], F32, tag="cs")
            zb = dtp.tile([HL, SB], F32, tag="zb")
            uu = dtp.tile([HL, SB], F32, tag="uu")
            pp = dtp.tile([HL, SB], F32, tag="pp")
            nc.scalar.activation(zb[:], dtraw[:], AF.Identity, bias=dtb_sb[:, 0:1])
            # uu = min(z, -z) = -|z|
            nc.vector.tensor_scalar(uu[:], zb[:], -1.0, None, ALU.mult)
            nc.vector.tensor_tensor(uu[:], uu[:], zb[:], ALU.min)
            nc.scalar.activation(uu[:], uu[:], AF.Exp)
            # Horner for q(u) = log1p(u)/u
            nc.vector.tensor_scalar(pp[:], uu[:], LOG1P_C[-1], LOG1P_C[-2],
                                    ALU.mult, ALU.add)
            for cidx in range(len(LOG1P_C) - 3, -1, -1):
                nc.vector.tensor_tensor(pp[:], pp[:], uu[:], ALU.mult)
                nc.vector.tensor_scalar(pp[:], pp[:], LOG1P_C[cidx], None, ALU.add)
            nc.vector.tensor_tensor(pp[:], pp[:], uu[:], ALU.mult)
            nc.scalar.activation(dtsp[:], zb[:], AF.Relu)
            nc.vector.tensor_tensor(dtsp[:], dtsp[:], pp[:], ALU.add)
            nc.vector.tensor_scalar(dtsp[:], dtsp[:], DT_MIN, DT_MAX, ALU.max, ALU.min)
            nc.vector.tensor_scalar(dA[:], dtsp[:], a_sb[:, 0:1], None, ALU.mult)
            for cl in range(NCPB):
                nc.vector.tensor_tensor_scan(
                    cs[:, cl * CHUNK:(cl + 1) * CHUNK],
                    ones16[:], dA[:, cl * CHUNK:(cl + 1) * CHUNK],
                    0.0, ALU.mult, ALU.add)
            nc.sync.dma_start(out=cs_dram[:, sb * SB:(sb + 1) * SB], in_=cs[:])

            # ---------------- conv + SiLU ----------------
            xc = convp.tile([128, 8 * SB], F32, tag="xc")
            bcs = convp.tile([128, SB], F32, tag="bc")
            ccs = convp.tile([128, SB], F32, tag="cc")
            for t in range(10):
                base = t * (SB + 3)
                dst = (xc[:, t * SB:(t + 1) * SB] if t < 8
                       else (bcs[:] if t == 8 else ccs[:]))
                nc.vector.tensor_scalar(
                    dst, ccat[:, base:base + SB],
                    cw_sb[:, t * KCONV:t * KCONV + 1], cb_sb[:, t:t + 1],
                    ALU.mult, ALU.add)
                for j in range(1, KCONV):
                    nc.vector.scalar_tensor_tensor(
                        dst, ccat[:, base + j:base + j + SB],
                        cw_sb[:, t * KCONV + j:t * KCONV + j + 1], dst,
                        ALU.mult, ALU.add)
                nc.scalar.activation(dst, dst, AF.Silu)

            # ---------------- SSD chunks ----------------
            for cl in range(NCPB):
                ch = sb * NCPB + cl
                csl = slice(cl * CHUNK, (cl + 1) * CHUNK)

                # small transposes: csT, dtT
                pT = psS.tile([128, 128], F32, tag="small")
                nc.tensor.transpose(pT[:, :HL], cs[:, csl], id_sb[:HL, :HL])
                csT = chp.tile([128, HL], F32, tag="csT")
                negcsT = chp.tile([128, HL], F32, tag="negcsT")
                ecsT = chp.tile([128, HL], F32, tag="ecsT")
                nc.scalar.copy(csT[:], pT[:, :HL])
                nc.scalar.mul(negcsT[:], pT[:, :HL], -1.0)
                nc.scalar.activation(ecsT[:], csT[:], AF.Exp)

                pT2 = psS.tile([128, 128], F32, tag="small")
                nc.tensor.transpose(pT2[:, :HL], dtsp[:, csl], id_sb[:HL, :HL])
                dtT = chp.tile([128, HL], F32, tag="dtT")
                nc.scalar.copy(dtT[:], pT2[:, :HL])

                cs127 = chp.tile([128, HL], F32, tag="cs127")
                cdbc = chp.tile([128, HL], F32, tag="cdbc")
                lastcol = sb * SB + cl * CHUNK + CHUNK - 1
                nc.sync.dma_start(
                    out=cs127[:],
                    in_=bc_ap(cs_dram[0:1, lastcol:lastcol + 1],
                              [[0, 128], [S, HL]]))
                nc.scalar.activation(cdbc[:], cs127[:], AF.Exp)
                decT = chp.tile([128, HL], F32, tag="decT")
                nc.vector.tensor_tensor(decT[:], cs127[:], csT[:], ALU.subtract)
                nc.scalar.activation(decT[:], decT[:], AF.Exp)
                ddt = chp.tile([128, HL], F32, tag="ddt")
                nc.vector.tensor_tensor(ddt[:], dtT[:], decT[:], ALU.mult)

                # x transpose -> xT, then xdt / xdd
                xps = psT.tile([128, CLOC], F32, tag="trans")
                for t in range(8):
                    nc.tensor.transpose(
                        xps[:, t * 128:(t + 1) * 128],
                        xc[:, t * SB + cl * CHUNK:t * SB + (cl + 1) * CHUNK],
                        id_sb[:])
                xT = chp.tile([128, CLOC], F32, tag="xT")
                nc.scalar.copy(xT[:], xps[:])
                xdt = chp.tile([128, CLOC], F32, tag="xdt")
                xdd = ch1p.tile([128, CLOC], F32, tag="xdd")
                for h in range(HL):
                    hs = slice(h * HD, (h + 1) * HD)
                    nc.vector.tensor_scalar(
                        xdt[:, hs], xT[:, hs], dtT[:, h:h + 1], None, ALU.mult)
                for h in range(HL):
                    hs = slice(h * HD, (h + 1) * HD)
                    nc.vector.tensor_scalar(
                        xdd[:, hs], xT[:, hs], ddt[:, h:h + 1], None, ALU.mult)

                # gate transpose + SiLU
                gps = psT.tile([128, CLOC], F32, tag="trans")
                for t in range(8):
                    nc.tensor.transpose(
                        gps[:, t * 128:(t + 1) * 128],
                        gate_sb[:, t * SB + cl * CHUNK:t * SB + (cl + 1) * CHUNK],
                        id_sb[:])
                silg = chp.tile([128, CLOC], F32, tag="silg")
                nc.scalar.activation(silg[:], gps[:], AF.Silu)

                # B chunk transposed (B_LN)
                pb_ps = psS.tile([128, 128], F32, tag="small")
                nc.tensor.transpose(pb_ps[:], bcs[:, csl], id_sb[:])
                bln = chp.tile([128, 128], F32, tag="bln")
                nc.scalar.copy(bln[:], pb_ps[:])

                # Yoff = C^T(prev state), scaled by exp(cs) at eviction
                yoff_ps = psY.tile([128, CLOC], F32, tag="yo")
                for half in range(2):
                    hsl = slice(half * 512, (half + 1) * 512)
                    nc.tensor.matmul(
                        yoff_ps[:, hsl], ccs[:, csl], st_sb[:, hsl],
                        start=True, stop=True)
                yoffs = ch1p.tile([128, CLOC], F32, tag="yoffs")
                for h in range(HL):
                    hs = slice(h * HD, (h + 1) * HD)
                    nc.vector.tensor_scalar(
                        yoffs[:, hs], yoff_ps[:, hs], ecsT[:, h:h + 1], None,
                        ALU.mult)

                # Gram^T = B C^T in [s, l]
                gram_ps = psS.tile([128, 128], F32, tag="small")
                nc.tensor.matmul(gram_ps[:], bcs[:, csl], ccs[:, csl],
                                 start=True, stop=True)

                # per-head masked decay matrices + Ydiag
                y_ps = psY.tile([128, CLOC], F32, tag="yo")
                for h in range(HL):
                    hs = slice(h * HD, (h + 1) * HD)
                    pb = hp.tile([128, 128], F32, tag="pb")
                    col0 = sb * SB + cl * CHUNK
                    nc.sync.dma_start(
                        out=pb[:],
                        in_=bc_ap(cs_dram[h:h + 1, col0:col0 + CHUNK],
                                  [[0, 128], [1, CHUNK]]))
                    seg = hp.tile([128, 128], F32, tag="seg")
                    nc.vector.scalar_tensor_tensor(
                        seg[:], pb[:], negcsT[:, h:h + 1], nm_sb[:],
                        ALU.add, ALU.add)
                    nc.scalar.activation(seg[:], seg[:], AF.Exp)
                    msk = hp.tile([128, 128], F32, tag="msk")
                    nc.vector.tensor_tensor(msk[:], seg[:], gram_ps[:], ALU.mult)
                    nc.tensor.matmul(y_ps[:, hs], msk[:], xdt[:, hs],
                                     start=True, stop=True)

                # states for this chunk
                s_ps = psY.tile([128, CLOC], F32, tag="yo")
                for half in range(2):
                    hsl = slice(half * 512, (half + 1) * 512)
                    nc.tensor.matmul(
                        s_ps[:, hsl], bln[:], xdd[:, hsl],
                        start=True, stop=True)

                # y = Ydiag + scaled Yoff ; state update
                y_sb = ch1p.tile([128, CLOC], F32, tag="ysb")
                nc.vector.tensor_tensor(y_sb[:], yoffs[:], y_ps[:], ALU.add)
                for h in range(HL):
                    hs = slice(h * HD, (h + 1) * HD)
                    nc.vector.scalar_tensor_tensor(
                        st_sb[:, hs], st_sb[:, hs], cdbc[:, h:h + 1], s_ps[:, hs],
                        ALU.mult, ALU.add)

                # + D * x
                for h in range(HL):
                    hs = slice(h * HD, (h + 1) * HD)
                    nc.vector.scalar_tensor_tensor(
                        y_sb[:, hs], xT[:, hs], dbc_sb[:, h:h + 1], y_sb[:, hs],
                        ALU.mult, ALU.add)

                # gate + group RMSNorm
                nc.vector.tensor_tensor(y_sb[:], y_sb[:], silg[:], ALU.mult)
                ssum = chp.tile([128, 1], F32, tag="ssum")
                # Square's main output is discarded into xdd (scratch)
                nc.scalar.activation(xdd[:], y_sb[:], AF.Square,
                                     accum_out=ssum[:, 0:1])
                nc.vector.tensor_scalar(ssum[:], ssum[:], 1.0 / GROUP, EPS,
                                        ALU.mult, ALU.add)
                rstd = chp.tile([128, 1], F32, tag="rstd")
                nc.scalar.activation(rstd[:], ssum[:], AF.Sqrt)
                rinv = chp.tile([128, 1], F32, tag="rinv")
                nc.vector.reciprocal(rinv[:], rstd[:])
                normed = ch1p.tile([128, CLOC], F32, tag="normed")
                nc.vector.scalar_tensor_tensor(
                    normed[:], y_sb[:], rinv[:, 0:1], nw_sb[:], ALU.mult, ALU.mult)

                # transpose normed -> [c, s] and stage out to DRAM
                nps = psT.tile([128, CLOC], F32, tag="trans")
                for t in range(8):
                    nc.tensor.transpose(
                        nps[:, t * 128:(t + 1) * 128],
                        normed[:, t * 128:(t + 1) * 128], id_sb[:])
                nstage = ch1p.tile([128, CLOC], F32, tag="nstage")
                nc.scalar.copy(nstage[:], nps[:])
                for t in range(8):
                    nc.sync.dma_start(
                        out=normT[t * 128:(t + 1) * 128,
                                  ch * CHUNK:(ch + 1) * CHUNK],
                        in_=nstage[:, t * 128:(t + 1) * 128])


def _out_proj_phase(nc, tc, w2t, normT, outp):
    with tc.tile_pool(name="ntile", bufs=1) as ntp, \
         tc.tile_pool(name="w2", bufs=6) as w2p, \
         tc.tile_pool(name="oev", bufs=3) as oevp, \
         tc.tile_pool(name="psO", bufs=4, space="PSUM") as psO:
        ntiles = []
        for kt in range(NK2):
            ntile = ntp.tile([128, S], F32R, tag=f"nt{kt}")
            nc.sync.dma_start(out=ntile[:], in_=r32(normT[kt * 128:(kt + 1) * 128, :]))
            ntiles.append(ntile)
        for m in range(NM2):
            for q in range(4):
                acc = psO.tile([128, 512], F32, tag="opacc")
                for kt in range(NK2):
                    w2 = w2p.tile([128, 128], F32R, tag="w2")
                    nc.sync.dma_start(out=w2[:], in_=r32(w2t[kt, m]))
                    nc.tensor.matmul(
                        acc[:], w2[:],
                        ntiles[kt][:, q * 512:(q + 1) * 512],
                        start=(kt == 0), stop=(kt == NK2 - 1))
                ev = oevp.tile([128, 512], F32, tag="oev")
                nc.scalar.copy(ev[:], acc[:])
                nc.sync.dma_start(
                    out=outp[m * 128:(m + 1) * 128, q * 512:(q + 1) * 512],
                    in_=ev[:])


def prepare_in_maps(hidden_states, in_proj_w, conv_w, conv_b, dt_bias, D,
                    norm_w, out_proj_w):
    hidT = np.ascontiguousarray(hidden_states.reshape(S, H_SIZE).T)
    negmask = np.where(np.arange(128)[None, :] >= np.arange(128)[:, None],
                       np.float32(0.0), np.float32(NEGM)).astype(np.float32)
    ident = np.eye(128, dtype=np.float32)
    in_maps = []
    for c in range(N_CORES):
        gsl = slice(CLOC * c, CLOC * (c + 1))
        xsl = slice(INTER + CLOC * c, INTER + CLOC * (c + 1))
        bsl = slice(2 * INTER + SS * c, 2 * INTER + SS * (c + 1))
        cslc = slice(2 * INTER + NG * SS + SS * c,
                     2 * INTER + NG * SS + SS * (c + 1))
        dsl = slice(INTER + CONV_DIM + HL * c, INTER + CONV_DIM + HL * (c + 1))
        w1 = np.concatenate([in_proj_w[gsl], in_proj_w[xsl], in_proj_w[bsl],
                             in_proj_w[cslc], in_proj_w[dsl]], axis=0)
        w1 = np.concatenate(
            [w1, np.zeros((NF * 128 - w1.shape[0], H_SIZE), np.float32)], axis=0)
        w1t = np.ascontiguousarray(
            w1.T.reshape(NK1, 128, NF, 128).transpose(0, 2, 1, 3))
        w2 = out_proj_w[:, gsl]  # [4096, 1024]
        w2t = np.ascontiguousarray(
            w2.T.reshape(NK2, 128, NM2, 128).transpose(0, 2, 1, 3))
        conv_idx = np.concatenate([
            np.arange(CLOC * c, CLOC * (c + 1)),
            np.arange(INTER + SS * c, INTER + SS * (c + 1)),
            np.arange(INTER + NG * SS + SS * c, INTER + NG * SS + SS * (c + 1))])
        cwl = conv_w[conv_idx, 0, :]          # [1280, 4]
        cbl = conv_b[conv_idx]                # [1280]
        convw = np.ascontiguousarray(
            cwl.reshape(10, 128, KCONV).transpose(1, 0, 2).reshape(128, 10 * KCONV))
        convb = np.ascontiguousarray(
            cbl.reshape(10, 128).transpose(1, 0))
        hsl = slice(HL * c, HL * (c + 1))
        acol = -(np.arange(HL * c + 1, HL * (c + 1) + 1, dtype=np.float32))
        in_maps.append({
            "hidT": hidT,
            "w1t": w1t,
            "w2t": w2t,
            "convw": convw,
            "convb": convb,
            "dtbias": dt_bias[hsl].reshape(HL, 1).astype(np.float32),
            "acol": acol.reshape(HL, 1),
            "dbc": np.tile(D[hsl][None, :], (128, 1)).astype(np.float32),
            "normwbc": np.tile(norm_w[gsl][None, :], (128, 1)).astype(np.float32),
            "negmask": negmask,
            "ident": ident,
        })
    return in_maps


def get_nc():
    if "nc" not in _CACHE:
        _CACHE["nc"] = build_nc()
    return _CACHE["nc"]


def kernel(hidden_states, in_proj_w, conv_w, conv_b, dt_bias, D, norm_w,
           out_proj_w):
    nc = get_nc()
    in_maps = prepare_in_maps(
        np.asarray(hidden_states, np.float32), np.asarray(in_proj_w, np.float32),
        np.asarray(conv_w, np.float32), np.asarray(conv_b, np.float32),
        np.asarray(dt_bias, np.float32), np.asarray(D, np.float32),
        np.asarray(norm_w, np.float32), np.asarray(out_proj_w, np.float32))
    res = run_bass_kernel_spmd(nc, in_maps, list(range(N_CORES)))
    acc = np.zeros((H_SIZE, S), np.float64)
    for r in res.results:
        acc += r["outp"].astype(np.float64)
    return acc.T.astype(np.float32).reshape(1, S, H_SIZE)


# revision 27
# speedup vs baseline: 1.0480x; 1.0480x over previous
"""NemotronH Mamba2 mixer on 8 Trainium2 cores (Bass/Tile).

Sharding: tensor-parallel over heads/groups. Core c owns group c =
16 heads (= 1024 gate/x channels, 128 B + 128 C state channels, 16 dt).
in_proj rows and out_proj columns are sharded accordingly; out_proj is
row-parallel over the contraction, partials are combined on the host.

Per-core dataflow (seq superblocks of 512, SSD chunks of 128):
  in_proj (fp32r matmul, weights pre-tiled for single-DMA loads)
  -> depthwise conv taps on DVE + SiLU
  -> Mamba2 chunked SSD: per-head decay matrices built with a PE
     broadcast matmul (indicator x cs), Ydiag + Yoff accumulated in one
     PSUM group per head
  -> gated group RMSNorm -> out_proj (fp32r) -> partial [4096, 2048].
"""

import numpy as np

import concourse.bass as bass
import concourse.mybir as mybir
from concourse import bacc
from concourse.tile import TileContext
from concourse.bass_utils import run_bass_kernel_spmd

F32 = mybir.dt.float32
F32R = mybir.dt.float32r
AF = mybir.ActivationFunctionType
ALU = mybir.AluOpType

# Model dims
H_SIZE = 4096
NH = 128
HD = 64
SS = 128
KCONV = 4
NG = 8
CHUNK = 128
INTER = NH * HD                 # 8192
CONV_DIM = INTER + 2 * NG * SS  # 10240
PROJ = INTER + CONV_DIM + NH    # 18560
DT_MIN, DT_MAX = 0.001, 100.0
EPS = 1e-5
GROUP = INTER // NG             # 1024

# Sharding / tiling
N_CORES = 8
S = 2048
HL = NH // N_CORES              # 16 local heads
CLOC = HL * HD                  # 1024 local gate/x channels
NSB = 4                         # seq superblocks
SB = S // NSB                   # 512
NCPB = SB // CHUNK              # 4 chunks per superblock
NCH = S // CHUNK                # 16 chunks
NF = 19                         # in_proj f-tiles (2432 = 19*128, padded)
NK1 = H_SIZE // 128             # 32 k-tiles for in_proj
NK2 = CLOC // 128               # 8 k-tiles for out_proj
NM2 = H_SIZE // 128             # 32 m-tiles for out_proj
NEGM = -1e30

# log1p(u)/u on [0,1], Chebyshev-fit degree 12 (max fp32 err ~1.1e-7)
LOG1P_C = [0.9999999999815061, -0.4999999935552795, 0.33333295899388315,
           -0.2499913901062215, 0.19989602251462296, -0.1659083573590588,
           0.1392317246686566, -0.1130135727826319, 0.08261769871302305,
           -0.04960969557400616, 0.021956439674455992, -0.006180556818034449,
           0.0008159022224092772]

_CACHE = {}


def r32(ap):
    return ap.bitcast(F32R)


def build_nc():
    nc = bacc.Bacc(None, target_bir_lowering=False)

    # hidden, pre-tiled: [sb, half, 128, 16*512] (per-partition contiguous)
    hids = nc.declare_dram_parameter("hids", [NSB, 2, 128, 16 * SB], F32,
                                     isOutput=False)
    # in_proj weights, pre-tiled per f-tile: [f, half, 128, 16*128]
    w1f = nc.declare_dram_parameter("w1f", [NF, 2, 128, 16 * 128], F32,
                                    isOutput=False)
    # out_proj weights, pre-tiled per m-tile: [m, 128, 8*128]
    w2m = nc.declare_dram_parameter("w2m", [NM2, 128, NK2 * 128], F32,
                                    isOutput=False)
    convw = nc.declare_dram_parameter("convw", [128, 10 * KCONV], F32,
                                      isOutput=False)
    convb = nc.declare_dram_parameter("convb", [128, 10], F32, isOutput=False)
    dtbias = nc.declare_dram_parameter("dtbias", [HL, 1], F32, isOutput=False)
    acol = nc.declare_dram_parameter("acol", [HL, 1], F32, isOutput=False)
    dbc = nc.declare_dram_parameter("dbc", [128, HL], F32, isOutput=False)
    negmask = nc.declare_dram_parameter("negmask", [128, 128], F32,
                                        isOutput=False)
    ident = nc.declare_dram_parameter("ident", [128, 128], F32, isOutput=False)
    e127 = nc.declare_dram_parameter("e127", [128, 1], F32, isOutput=False)
    outp = nc.declare_dram_parameter("outp", [NM2, 4, 128, 512], F32,
                                     isOutput=True)

    with TileContext(nc) as tc:
        with tc.tile_pool(name="const", bufs=1) as cp, \
             tc.tile_pool(name="dram", bufs=1, space="DRAM") as dp:
            id_sb = cp.tile([128, 128], F32, tag="id")
            nm_sb = cp.tile([128, 128], F32, tag="nm")
            dbc_sb = cp.tile([128, HL], F32, tag="dbc")
            cw_sb = cp.tile([128, 10 * KCONV], F32, tag="cw")
            cb_sb = cp.tile([128, 10], F32, tag="cb")
            dtb_sb = cp.tile([HL, 1], F32, tag="dtb")
            a_sb = cp.tile([HL, 1], F32, tag="acol")
            e127_sb = cp.tile([128, 1], F32, tag="e127")
            ones16 = cp.tile([HL, CHUNK], F32, tag="ones16")
            st_sb = cp.tile([128, HL * HD], F32, tag="state")
            nc.sync.dma_start(out=id_sb[:], in_=ident[:])
            nc.sync.dma_start(out=nm_sb[:], in_=negmask[:])
            nc.sync.dma_start(out=dbc_sb[:], in_=dbc[:])
            nc.sync.dma_start(out=cw_sb[:], in_=convw[:])
            nc.sync.dma_start(out=cb_sb[:], in_=convb[:])
            nc.sync.dma_start(out=dtb_sb[:], in_=dtbias[:])
            nc.sync.dma_start(out=a_sb[:], in_=acol[:])
            nc.sync.dma_start(out=e127_sb[:], in_=e127[:])
            nc.vector.memset(ones16[:], 1.0)
            nc.vector.memset(st_sb[:], 0.0)

            _main_phase(nc, tc, hids, w1f, id_sb, nm_sb, dbc_sb,
                        cw_sb, cb_sb, dtb_sb, a_sb, e127_sb, ones16,
                        st_sb, w2m, outp)

    nc.compile()
    return nc


def _main_phase(nc, tc, hids, w1f, id_sb, nm_sb, dbc_sb,
                cw_sb, cb_sb, dtb_sb, a_sb, e127_sb, ones16,
                st_sb, w2m, outp):
    with tc.tile_pool(name="hid", bufs=2) as hidp, \
         tc.tile_pool(name="w1", bufs=3) as w1p, \
         tc.tile_pool(name="gq", bufs=2) as gqp, \
         tc.tile_pool(name="conv", bufs=1) as convp, \
         tc.tile_pool(name="dtl", bufs=1) as dtp, \
         tc.tile_pool(name="dtr", bufs=2) as dtrp, \
         tc.tile_pool(name="cch", bufs=2) as cchp, \
         tc.tile_pool(name="chunk", bufs=2) as chp, \
         tc.tile_pool(name="chunk1", bufs=1) as ch1p, \
         tc.tile_pool(name="w2", bufs=2) as w2p, \
         tc.tile_pool(name="oev", bufs=2) as oevp, \
         tc.tile_pool(name="heads", bufs=1) as hp, \
         tc.tile_pool(name="psA", bufs=1, space="PSUM") as psA, \
         tc.tile_pool(name="psS", bufs=1, space="PSUM") as psS, \
         tc.tile_pool(name="psT", bufs=1, space="PSUM") as psT, \
         tc.tile_pool(name="psY", bufs=2, space="PSUM") as psY:

        # convcat: 10 conv channel tiles (8 x, 1 B, 1 C), each 3 halo + 512
        ccat = convp.tile([128, 10 * (SB + 3)], F32, tag="ccat")
        for t in range(10):
            nc.vector.memset(ccat[:, t * (SB + 3):t * (SB + 3) + 3], 0.0)

        pending_out = []

        def emit_outproj(m, qst, sbq):
            w2 = w2p.tile([128, NK2 * 128], F32R, tag="w2")
            nc.sync.dma_start(out=w2[:], in_=r32(w2m[m]))
            acc = psA.tile([128, 512], F32, tag="ipacc")
            for kt in range(NK2):
                nc.tensor.matmul(
                    acc[:], w2[:, kt * 128:(kt + 1) * 128],
                    qst[:, kt * SB:kt * SB + SB],
                    start=(kt == 0), stop=(kt == NK2 - 1))
            ev = oevp.tile([128, 512], F32, tag="oev")
            nc.scalar.copy(ev[:], acc[:])
            nc.sync.dma_start(out=outp[m, sbq], in_=ev[:])

        for sb in range(NSB):
            # ---------------- in_proj for this superblock ----------------
            halves = []
            for khalf in range(2):
                hid_h = hidp.tile([128, 16 * SB], F32R, tag="hid")
                nc.sync.dma_start(out=hid_h[:], in_=r32(hids[sb, khalf]))
                halves.append(hid_h)

            gate_sb = gqp.tile([128, 8 * SB], F32, tag="gq")
            dtraw = dtrp.tile([HL, SB], F32, tag="dtraw")

            # halo copies must read previous superblock before overwrite
            if sb > 0:
                for t in range(10):
                    base = t * (SB + 3)
                    nc.vector.tensor_copy(
                        ccat[:, base:base + 3], ccat[:, base + SB:base + SB + 3])

            def emit_ftile(f, gate_sb=gate_sb, dtraw=dtraw, halves=halves):
                w1h = []
                for khalf in range(2):
                    w1t_ = w1p.tile([128, 16 * 128], F32R, tag="w1")
                    nc.sync.dma_start(out=w1t_[:], in_=r32(w1f[f, khalf]))
                    w1h.append(w1t_)
                acc = psA.tile([128, SB], F32, tag="ipacc")
                for k in range(NK1):
                    nc.tensor.matmul(
                        acc[:],
                        w1h[k // 16][:, (k % 16) * 128:(k % 16 + 1) * 128],
                        halves[k // 16][:, (k % 16) * SB:(k % 16 + 1) * SB],
                        start=(k == 0), stop=(k == NK1 - 1))
                if f < 8:
                    nc.scalar.copy(gate_sb[:, f * SB:(f + 1) * SB], acc[:])
                elif f < 18:
                    t = f - 8
                    base = t * (SB + 3)
                    nc.scalar.copy(ccat[:, base + 3:base + 3 + SB], acc[:])
                else:
                    nc.scalar.copy(dtraw[:, :], acc[:HL, :])

            for f in [18] + list(range(8, 18)) + list(range(8)):
                emit_ftile(f)
                for _ in range(2):
                    if pending_out:
                        emit_outproj(*pending_out.pop(0))

            nc.scalar.activation(gate_sb[:], gate_sb[:], AF.Silu)

            # ---------------- dt pipeline ----------------
            # softplus(z) = relu(z) + log1p(exp(-|z|)); log1p via poly
            # (no Softplus/Ln activation table on gen3)
            uu = dtp.tile([HL, SB], F32, tag="uu")
            pp = dtrp.tile([HL, SB], F32, tag="pp")
            dtsp = dtraw  # in-place: relu(z) overwrites z
            cs = pp       # reuse pp once the poly is folded in
            nc.scalar.activation(dtraw[:], dtraw[:], AF.Identity,
                                 bias=dtb_sb[:, 0:1])
            # uu = exp(min(z, -z)) = exp(-|z|)
            nc.vector.tensor_scalar(uu[:], dtraw[:], -1.0, None, ALU.mult)
            nc.vector.tensor_tensor(uu[:], uu[:], dtraw[:], ALU.min)
            nc.scalar.activation(uu[:], uu[:], AF.Exp)
            # Horner for q(u) = log1p(u)/u
            nc.vector.tensor_scalar(pp[:], uu[:], LOG1P_C[-1], LOG1P_C[-2],
                                    ALU.mult, ALU.add)
            for cidx in range(len(LOG1P_C) - 3, -1, -1):
                nc.vector.tensor_tensor(pp[:], pp[:], uu[:], ALU.mult)
                nc.vector.tensor_scalar(pp[:], pp[:], LOG1P_C[cidx], None,
                                        ALU.add)
            nc.vector.tensor_tensor(pp[:], pp[:], uu[:], ALU.mult)
            relu_t = uu  # uu dead; use as relu scratch
            nc.scalar.activation(relu_t[:], dtraw[:], AF.Relu)
            nc.vector.tensor_tensor(dtsp[:], relu_t[:], pp[:], ALU.add)
            nc.vector.tensor_scalar(dtsp[:], dtsp[:], DT_MIN, DT_MAX,
                                    ALU.max, ALU.min)
            dA = uu  # reuse again (relu scratch is dead)
            nc.vector.tensor_scalar(dA[:], dtsp[:], a_sb[:, 0:1], None,
                                    ALU.mult)
            for cl in range(NCPB):
                nc.vector.tensor_tensor_scan(
                    cs[:, cl * CHUNK:(cl + 1) * CHUNK],
                    ones16[:], dA[:, cl * CHUNK:(cl + 1) * CHUNK],
                    0.0, ALU.mult, ALU.add)

            # ---------------- SSD chunks ----------------
            qstage = gqp.tile([128, NK2 * SB], F32R, tag="gq")

            def emit_conv(cl):
                xc = cchp.tile([128, 8 * CHUNK], F32, tag="xc")
                bcs = ch1p.tile([128, CHUNK], F32, tag="bc")
                ccs = ch1p.tile([128, CHUNK], F32, tag="cc")
                for t in range(10):
                    base = t * (SB + 3) + cl * CHUNK
                    dst = (xc[:, t * CHUNK:(t + 1) * CHUNK] if t < 8
                           else (bcs[:] if t == 8 else ccs[:]))
                    nc.vector.tensor_scalar(
                        dst, ccat[:, base:base + CHUNK],
                        cw_sb[:, t * KCONV:t * KCONV + 1], cb_sb[:, t:t + 1],
                        ALU.mult, ALU.add)
                    for j in range(1, KCONV):
                        nc.vector.scalar_tensor_tensor(
                            dst, ccat[:, base + j:base + j + CHUNK],
                            cw_sb[:, t * KCONV + j:t * KCONV + j + 1], dst,
                            ALU.mult, ALU.add)
                    nc.scalar.activation(dst, dst, AF.Silu)
                return xc, bcs, ccs

            convs = [emit_conv(cl) for cl in range(NCPB)]
            for cl in range(NCPB):
                ch = sb * NCPB + cl
                csl = slice(cl * CHUNK, (cl + 1) * CHUNK)
                xc, bcs, ccs = convs[cl]

                # gate transpose + SiLU
                gps = psT.tile([128, CLOC], F32, tag="trans")
                for t in range(8):
                    nc.tensor.transpose(
                        gps[:, t * 128:(t + 1) * 128],
                        gate_sb[:, t * SB + cl * CHUNK:t * SB + (cl + 1) * CHUNK],
                        id_sb[:])
                silg = ch1p.tile([128, CLOC], F32, tag="silg")
                nc.scalar.copy(silg[:], gps[:])

                # small transposes: csT, dtT
                pT = psS.tile([128, 128], F32, tag="small")
                nc.tensor.transpose(pT[:, :HL], cs[:, csl], id_sb[:HL, :HL])
                csT = chp.tile([128, HL], F32, tag="csT")
                negcsT = chp.tile([128, HL], F32, tag="negcsT")
                nc.scalar.copy(csT[:], pT[:, :HL])
                nc.scalar.mul(negcsT[:], pT[:, :HL], -1.0)

                pT2 = psS.tile([128, 128], F32, tag="small")
                nc.tensor.transpose(pT2[:, :HL], dtsp[:, csl], id_sb[:HL, :HL])
                dtT = chp.tile([128, HL], F32, tag="dtT")
                nc.scalar.copy(dtT[:], pT2[:, :HL])

                # cs at chunk end, broadcast across partitions (PE matmul)
                pT3 = psS.tile([128, 128], F32, tag="small")
                e127b = bass.AP(tensor=e127_sb.tensor,
                                offset=e127_sb[:].offset,
                                ap=[[e127_sb[:].ap[0][0], 128], [0, 128]])
                nc.tensor.matmul(pT3[:, :HL], e127b, csT[:],
                                 start=True, stop=True)
                cdbc = chp.tile([128, HL], F32, tag="cdbc")
                decT = chp.tile([128, HL], F32, tag="decT")
                nc.scalar.activation(cdbc[:], pT3[:, :HL], AF.Exp)
                nc.vector.tensor_tensor(decT[:], pT3[:, :HL], csT[:],
                                        ALU.subtract)
                nc.scalar.activation(decT[:], decT[:], AF.Exp)
                ddt = chp.tile([128, HL], F32, tag="ddt")
                nc.vector.tensor_tensor(ddt[:], dtT[:], decT[:], ALU.mult)

                # x transpose -> xT, then xdt / xdd
                xps = psT.tile([128, CLOC], F32, tag="trans")
                for t in range(8):
                    nc.tensor.transpose(
                        xps[:, t * 128:(t + 1) * 128],
                        xc[:, t * CHUNK:(t + 1) * CHUNK], id_sb[:])
                xT = ch1p.tile([128, CLOC], F32, tag="xT")
                nc.scalar.copy(xT[:], xps[:])
                xdt = ch1p.tile([128, CLOC], F32, tag="xdt")
                xdd = ch1p.tile([128, CLOC], F32R, tag="xdd")
                for h in range(HL):
                    hs = slice(h * HD, (h + 1) * HD)
                    nc.vector.tensor_scalar(
                        xdt[:, hs], xT[:, hs], dtT[:, h:h + 1], None, ALU.mult)
                for h in range(HL):
                    hs = slice(h * HD, (h + 1) * HD)
                    nc.vector.tensor_scalar(
                        xdd[:, hs], xT[:, hs], ddt[:, h:h + 1], None, ALU.mult)

                # B chunk transposed (B_LN)
                pbt = psS.tile([128, 128], F32, tag="small")
                nc.tensor.transpose(pbt[:], bcs[:], id_sb[:])
                bln = chp.tile([128, 128], F32R, tag="bln")
                nc.scalar.copy(bln[:], pbt[:])

                # Gram^T = B C^T in [s, l]; evicted to SBUF
                gram_ps = psS.tile([128, 128], F32, tag="small")
                nc.tensor.matmul(gram_ps[:], bcs[:], ccs[:],
                                 start=True, stop=True)
                gram = ch1p.tile([128, 128], F32, tag="gram")
                nc.scalar.copy(gram[:], gram_ps[:])

                # per-head decay matrices in groups of 4 heads
                y_ps = psY.tile([128, CLOC], F32, tag="yo")
                for g in range(HL // 4):
                    pb4 = psS.tile([128, 512], F32, tag="small")
                    for j in range(4):
                        h = 4 * g + j
                        idcol = id_sb[:HL, h:h + 1]
                        indh = bass.AP(tensor=idcol.tensor,
                                       offset=idcol.offset,
                                       ap=[[idcol.ap[0][0], HL], [0, 128]])
                        nc.tensor.matmul(pb4[:, j * 128:(j + 1) * 128], indh,
                                         cs[:, csl], start=True, stop=True)
                    epb4 = hp.tile([128, 512], F32, tag="epb")
                    nc.scalar.activation(epb4[:], pb4[:], AF.Exp)
                    seg4 = hp.tile([128, 512], F32, tag="seg")
                    for j in range(4):
                        h = 4 * g + j
                        nc.vector.scalar_tensor_tensor(
                            seg4[:, j * 128:(j + 1) * 128],
                            pb4[:, j * 128:(j + 1) * 128],
                            negcsT[:, h:h + 1], nm_sb[:], ALU.add, ALU.add)
                    nc.scalar.activation(seg4[:], seg4[:], AF.Exp)
                    gram_b = bass.AP(tensor=gram.tensor, offset=gram[:].offset,
                                     ap=[gram[:].ap[0], [0, 4], [1, 128]])
                    ccs_b = bass.AP(tensor=ccs.tensor, offset=ccs[:].offset,
                                    ap=[ccs[:].ap[0], [0, 4], [1, 128]])
                    s4 = seg4[:].rearrange("p (j l) -> p j l", j=4)
                    e4 = epb4[:].rearrange("p (j l) -> p j l", j=4)
                    nc.vector.tensor_tensor(s4, s4, gram_b, ALU.mult)
                    nc.vector.tensor_tensor(e4, e4, ccs_b, ALU.mult)
                    for j in range(4):
                        h = 4 * g + j
                        hs = slice(h * HD, (h + 1) * HD)
                        nc.tensor.matmul(
                            y_ps[:, hs], seg4[:, j * 128:(j + 1) * 128],
                            xdt[:, hs], start=True, stop=False)
                        nc.tensor.matmul(
                            y_ps[:, hs], epb4[:, j * 128:(j + 1) * 128],
                            st_sb[:, hs], start=False, stop=True)

                # states for this chunk
                s_ps = psY.tile([128, CLOC], F32, tag="yo")
                for half in range(2):
                    hsl = slice(half * 512, (half + 1) * 512)
                    nc.tensor.matmul(
                        s_ps[:, hsl], bln[:], xdd[:, hsl],
                        start=True, stop=True)

                # y = (Ydiag + Yoff) + D*x ; state update
                y_sb = ch1p.tile([128, CLOC], F32, tag="ysb")
                for h in range(HL):
                    hs = slice(h * HD, (h + 1) * HD)
                    nc.vector.scalar_tensor_tensor(
                        y_sb[:, hs], xT[:, hs], dbc_sb[:, h:h + 1],
                        y_ps[:, hs], ALU.mult, ALU.add)
                for h in range(HL):
                    hs = slice(h * HD, (h + 1) * HD)
                    nc.vector.scalar_tensor_tensor(
                        st_sb[:, hs], st_sb[:, hs], cdbc[:, h:h + 1],
                        s_ps[:, hs], ALU.mult, ALU.add)

                # gate + group RMSNorm
                nc.vector.tensor_tensor(y_sb[:], y_sb[:], silg[:], ALU.mult)
                ssum = ch1p.tile([128, 1], F32, tag="ssum")
                # Square's main output is discarded into xdd (scratch)
                nc.scalar.activation(xdd[:], y_sb[:], AF.Square,
                                     accum_out=ssum[:, 0:1])
                nc.vector.tensor_scalar(ssum[:], ssum[:], 1.0 / GROUP, EPS,
                                        ALU.mult, ALU.add)
                rstd = chp.tile([128, 1], F32, tag="rstd")
                tnew = chp.tile([128, 1], F32, tag="tnew")
                nc.scalar.activation(tnew[:], ssum[:], AF.Sqrt)
                nc.vector.reciprocal(rstd[:], tnew[:])
                normed = ch1p.tile([128, CLOC], F32, tag="normed")
                nc.vector.tensor_scalar(
                    normed[:], y_sb[:], rstd[:, 0:1], None, ALU.mult)

                # transpose normed -> [c, s] and stage out to DRAM
                nps = psT.tile([128, CLOC], F32, tag="trans")
                for t in range(8):
                    nc.tensor.transpose(
                        nps[:, t * 128:(t + 1) * 128],
                        normed[:, t * 128:(t + 1) * 128], id_sb[:])
                qdst = qstage[:].rearrange(
                    "p (t s) -> p t s", t=NK2)[:, :, cl * 128:(cl + 1) * 128]
                nsrc = nps[:].rearrange("p (t s) -> p t s", t=NK2)
                nc.scalar.copy(qdst, nsrc)

            # out_proj m-blocks are deferred and interleaved into the
            # next superblock's in_proj f-loop (shared psA rotation)
            pending_out.extend((m, qstage, sb) for m in range(NM2))

        while pending_out:
            emit_outproj(*pending_out.pop(0))


def prepare_in_maps(hidden_states, in_proj_w, conv_w, conv_b, dt_bias, D,
                    norm_w, out_proj_w):
    hidT = np.ascontiguousarray(hidden_states.reshape(S, H_SIZE).T)
    # [half, kk, r, sb, c] -> [sb, half, r, kk, c]
    hids = np.ascontiguousarray(
        hidT.reshape(2, 16, 128, NSB, SB).transpose(3, 0, 2, 1, 4)
        .reshape(NSB, 2, 128, 16 * SB))
    negmask = np.where(np.arange(128)[None, :] >= np.arange(128)[:, None],
                       np.float32(0.0), np.float32(NEGM)).astype(np.float32)
    ident = np.eye(128, dtype=np.float32)
    e127 = np.zeros((128, 1), np.float32)
    e127[127, 0] = 1.0
    in_maps = []
    for c in range(N_CORES):
        gsl = slice(CLOC * c, CLOC * (c + 1))
        xsl = slice(INTER + CLOC * c, INTER + CLOC * (c + 1))
        bsl = slice(2 * INTER + SS * c, 2 * INTER + SS * (c + 1))
        cslc = slice(2 * INTER + NG * SS + SS * c,
                     2 * INTER + NG * SS + SS * (c + 1))
        dsl = slice(INTER + CONV_DIM + HL * c, INTER + CONV_DIM + HL * (c + 1))
        w1 = np.concatenate([in_proj_w[gsl], in_proj_w[xsl], in_proj_w[bsl],
                             in_proj_w[cslc], in_proj_w[dsl]], axis=0)
        w1 = np.concatenate(
            [w1, np.zeros((NF * 128 - w1.shape[0], H_SIZE), np.float32)],
            axis=0)
        # W1T [4096, 2432]: [half, kk, r, f, fc] -> [f, half, r, kk, fc]
        w1f = np.ascontiguousarray(
            w1.T.reshape(2, 16, 128, NF, 128).transpose(3, 0, 2, 1, 4)
            .reshape(NF, 2, 128, 16 * 128))
        w2 = out_proj_w[:, gsl] * norm_w[gsl][None, :]  # norm_w folded
        # W2T [1024, 4096]: [kt, r, m, mc] -> [m, r, kt, mc]
        w2m = np.ascontiguousarray(
            w2.T.reshape(NK2, 128, NM2, 128).transpose(2, 1, 0, 3)
            .reshape(NM2, 128, NK2 * 128))
        conv_idx = np.concatenate([
            np.arange(CLOC * c, CLOC * (c + 1)),
            np.arange(INTER + SS * c, INTER + SS * (c + 1)),
            np.arange(INTER + NG * SS + SS * c,
                      INTER + NG * SS + SS * (c + 1))])
        cwl = conv_w[conv_idx, 0, :]          # [1280, 4]
        cbl = conv_b[conv_idx]                # [1280]
        convw = np.ascontiguousarray(
            cwl.reshape(10, 128, KCONV).transpose(1, 0, 2)
            .reshape(128, 10 * KCONV))
        convb = np.ascontiguousarray(cbl.reshape(10, 128).transpose(1, 0))
        hsl = slice(HL * c, HL * (c + 1))
        acol = -(np.arange(HL * c + 1, HL * (c + 1) + 1, dtype=np.float32))
        in_maps.append({
            "hids": hids,
            "w1f": w1f,
            "w2m": w2m,
            "convw": convw,
            "convb": convb,
            "dtbias": dt_bias[hsl].reshape(HL, 1).astype(np.float32),
            "acol": acol.reshape(HL, 1),
            "dbc": np.tile(D[hsl][None, :], (128, 1)).astype(np.float32),
            "negmask": negmask,
            "ident": ident,
            "e127": e127,
        })
    return in_maps


def get_nc():
    if "nc" not in _CACHE:
        _CACHE["nc"] = build_nc()
    return _CACHE["nc"]


def kernel(hidden_states, in_proj_w, conv_w, conv_b, dt_bias, D, norm_w,
           out_proj_w):
    nc = get_nc()
    in_maps = prepare_in_maps(
        np.asarray(hidden_states, np.float32),
        np.asarray(in_proj_w, np.float32),
        np.asarray(conv_w, np.float32), np.asarray(conv_b, np.float32),
        np.asarray(dt_bias, np.float32), np.asarray(D, np.float32),
        np.asarray(norm_w, np.float32), np.asarray(out_proj_w, np.float32))
    res = run_bass_kernel_spmd(nc, in_maps, list(range(N_CORES)))
    acc = np.zeros((H_SIZE, S), np.float64)
    for r in res.results:
        acc += r["outp"].transpose(0, 2, 1, 3).reshape(H_SIZE, S)
    return acc.T.astype(np.float32).reshape(1, S, H_SIZE)


# revision 31
# speedup vs baseline: 113.4238x; 108.2247x over previous
"""NemotronH Mamba2 mixer on 8 Trainium2 cores (Bass/Tile).

Sharding: tensor-parallel over heads/groups. Core c owns group c =
16 heads (= 1024 gate/x channels, 128 B + 128 C state channels, 16 dt).
in_proj rows and out_proj columns are sharded accordingly; out_proj is
row-parallel over the contraction, partials are combined on the host.

Per-core dataflow (seq superblocks of 512, SSD chunks of 128):
  in_proj (fp32r matmul, weights pre-tiled for single-DMA loads)
  -> depthwise conv taps on DVE + SiLU
  -> Mamba2 chunked SSD: per-head decay matrices built with a PE
     broadcast matmul (indicator x cs), Ydiag + Yoff accumulated in one
     PSUM group per head
  -> gated group RMSNorm -> out_proj (fp32r) -> partial [4096, 2048].
"""

import numpy as np

import concourse.bass as bass
import concourse.mybir as mybir
from concourse import bacc
from concourse.tile import TileContext
from concourse.bass_utils import run_bass_kernel_spmd

F32 = mybir.dt.float32
F32R = mybir.dt.float32r
AF = mybir.ActivationFunctionType
ALU = mybir.AluOpType

# Model dims
H_SIZE = 4096
NH = 128
HD = 64
SS = 128
KCONV = 4
NG = 8
CHUNK = 128
INTER = NH * HD                 # 8192
CONV_DIM = INTER + 2 * NG * SS  # 10240
PROJ = INTER + CONV_DIM + NH    # 18560
DT_MIN, DT_MAX = 0.001, 100.0
EPS = 1e-5
GROUP = INTER // NG             # 1024

# Sharding / tiling
N_CORES = 8
S = 2048
HL = NH // N_CORES              # 16 local heads
CLOC = HL * HD                  # 1024 local gate/x channels
NSB = 4                         # seq superblocks
SB = S // NSB                   # 512
NCPB = SB // CHUNK              # 4 chunks per superblock
NCH = S // CHUNK                # 16 chunks
NF = 19                         # in_proj f-tiles (2432 = 19*128, padded)
NK1 = H_SIZE // 128             # 32 k-tiles for in_proj
NK2 = CLOC // 128               # 8 k-tiles for out_proj
NM2 = H_SIZE // 128             # 32 m-tiles for out_proj
NEGM = -1e30

# log1p(u)/u on [0,1], Chebyshev-fit degree 12 (max fp32 err ~1.1e-7)
LOG1P_C = [0.9999999999815061, -0.4999999935552795, 0.33333295899388315,
           -0.2499913901062215, 0.19989602251462296, -0.1659083573590588,
           0.1392317246686566, -0.1130135727826319, 0.08261769871302305,
           -0.04960969557400616, 0.021956439674455992, -0.006180556818034449,
           0.0008159022224092772]

_CACHE = {}


def r32(ap):
    return ap.bitcast(F32R)


def build_nc():
    nc = bacc.Bacc(None, target_bir_lowering=False)

    # hidden, pre-tiled: [sb, half, 128, 16*512] (per-partition contiguous)
    hids = nc.declare_dram_parameter("hids", [NSB, 2, 128, 16 * SB], F32,
                                     isOutput=False)
    # in_proj weights, pre-tiled per f-tile: [f, half, 128, 16*128]
    w1f = nc.declare_dram_parameter("w1f", [NF, 2, 128, 16 * 128], F32,
                                    isOutput=False)
    # out_proj weights, pre-tiled per m-tile: [m, 128, 8*128]
    w2m = nc.declare_dram_parameter("w2m", [NM2, 128, NK2 * 128], F32,
                                    isOutput=False)
    convw = nc.declare_dram_parameter("convw", [128, 10 * KCONV], F32,
                                      isOutput=False)
    convb = nc.declare_dram_parameter("convb", [128, 10], F32, isOutput=False)
    dtbias = nc.declare_dram_parameter("dtbias", [HL, 1], F32, isOutput=False)
    acol = nc.declare_dram_parameter("acol", [HL, 1], F32, isOutput=False)
    dbc = nc.declare_dram_parameter("dbc", [128, HL], F32, isOutput=False)
    negmask = nc.declare_dram_parameter("negmask", [128, 128], F32,
                                        isOutput=False)
    ident = nc.declare_dram_parameter("ident", [128, 128], F32, isOutput=False)
    e127 = nc.declare_dram_parameter("e127", [128, 1], F32, isOutput=False)
    outp = nc.declare_dram_parameter("outp", [NM2, 4, 128, 512], F32,
                                     isOutput=True)

    with TileContext(nc) as tc:
        with tc.tile_pool(name="const", bufs=1) as cp, \
             tc.tile_pool(name="dram", bufs=1, space="DRAM") as dp:
            id_sb = cp.tile([128, 128], F32, tag="id")
            nm_sb = cp.tile([128, 128], F32, tag="nm")
            dbc_sb = cp.tile([128, HL], F32, tag="dbc")
            cw_sb = cp.tile([128, 10 * KCONV], F32, tag="cw")
            cb_sb = cp.tile([128, 10], F32, tag="cb")
            dtb_sb = cp.tile([HL, 1], F32, tag="dtb")
            a_sb = cp.tile([HL, 1], F32, tag="acol")
            e127_sb = cp.tile([128, 1], F32, tag="e127")
            ones16 = cp.tile([HL, CHUNK], F32, tag="ones16")
            st_sb = cp.tile([128, HL * HD], F32, tag="state")
            nc.sync.dma_start(out=id_sb[:], in_=ident[:])
            nc.sync.dma_start(out=nm_sb[:], in_=negmask[:])
            nc.sync.dma_start(out=dbc_sb[:], in_=dbc[:])
            nc.sync.dma_start(out=cw_sb[:], in_=convw[:])
            nc.sync.dma_start(out=cb_sb[:], in_=convb[:])
            nc.sync.dma_start(out=dtb_sb[:], in_=dtbias[:])
            nc.sync.dma_start(out=a_sb[:], in_=acol[:])
            nc.sync.dma_start(out=e127_sb[:], in_=e127[:])
            nc.vector.memset(ones16[:], 1.0)
            nc.vector.memset(st_sb[:], 0.0)

            _main_phase(nc, tc, hids, w1f, id_sb, nm_sb, dbc_sb,
                        cw_sb, cb_sb, dtb_sb, a_sb, e127_sb, ones16,
                        st_sb, w2m, outp)

    nc.compile()
    return nc


def _main_phase(nc, tc, hids, w1f, id_sb, nm_sb, dbc_sb,
                cw_sb, cb_sb, dtb_sb, a_sb, e127_sb, ones16,
                st_sb, w2m, outp):
    with tc.tile_pool(name="hid", bufs=2) as hidp, \
         tc.tile_pool(name="w1", bufs=3) as w1p, \
         tc.tile_pool(name="gq", bufs=2) as gqp, \
         tc.tile_pool(name="conv", bufs=1) as convp, \
         tc.tile_pool(name="dtl", bufs=1) as dtp, \
         tc.tile_pool(name="dtr", bufs=2) as dtrp, \
         tc.tile_pool(name="cch", bufs=2) as cchp, \
         tc.tile_pool(name="chunk", bufs=2) as chp, \
         tc.tile_pool(name="chunk1", bufs=1) as ch1p, \
         tc.tile_pool(name="w2", bufs=2) as w2p, \
         tc.tile_pool(name="oev", bufs=2) as oevp, \
         tc.tile_pool(name="heads", bufs=1) as hp, \
         tc.tile_pool(name="psA", bufs=1, space="PSUM") as psA, \
         tc.tile_pool(name="psS", bufs=1, space="PSUM") as psS, \
         tc.tile_pool(name="psT", bufs=1, space="PSUM") as psT, \
         tc.tile_pool(name="psY", bufs=2, space="PSUM") as psY:

        # convcat: 10 conv channel tiles (8 x, 1 B, 1 C), each 3 halo + 512
        ccat = convp.tile([128, 10 * (SB + 3)], F32, tag="ccat")
        for t in range(10):
            nc.vector.memset(ccat[:, t * (SB + 3):t * (SB + 3) + 3], 0.0)

        pending_out = []

        def emit_outproj(m, qst, sbq):
            w2 = w2p.tile([128, NK2 * 128], F32R, tag="w2")
            nc.sync.dma_start(out=w2[:], in_=r32(w2m[m]))
            acc = psA.tile([128, 512], F32, tag="ipacc")
            for kt in range(NK2):
                nc.tensor.matmul(
                    acc[:], w2[:, kt * 128:(kt + 1) * 128],
                    qst[:, kt * SB:kt * SB + SB],
                    start=(kt == 0), stop=(kt == NK2 - 1))
            ev = oevp.tile([128, 512], F32, tag="oev")
            nc.scalar.copy(ev[:], acc[:])
            nc.sync.dma_start(out=outp[m, sbq], in_=ev[:])

        for sb in range(NSB):
            # ---------------- in_proj for this superblock ----------------
            halves = []
            for khalf in range(2):
                hid_h = hidp.tile([128, 16 * SB], F32R, tag="hid")
                nc.sync.dma_start(out=hid_h[:], in_=r32(hids[sb, khalf]))
                halves.append(hid_h)

            gate_sb = gqp.tile([128, 8 * SB], F32, tag="gq")
            dtraw = dtrp.tile([HL, SB], F32, tag="dtraw")

            # halo copies must read previous superblock before overwrite
            if sb > 0:
                for t in range(10):
                    base = t * (SB + 3)
                    nc.vector.tensor_copy(
                        ccat[:, base:base + 3], ccat[:, base + SB:base + SB + 3])

            def emit_ftile(f, gate_sb=gate_sb, dtraw=dtraw, halves=halves):
                w1h = []
                for khalf in range(2):
                    w1t_ = w1p.tile([128, 16 * 128], F32R, tag="w1")
                    nc.sync.dma_start(out=w1t_[:], in_=r32(w1f[f, khalf]))
                    w1h.append(w1t_)
                acc = psA.tile([128, SB], F32, tag="ipacc")
                for k in range(NK1):
                    nc.tensor.matmul(
                        acc[:],
                        w1h[k // 16][:, (k % 16) * 128:(k % 16 + 1) * 128],
                        halves[k // 16][:, (k % 16) * SB:(k % 16 + 1) * SB],
                        start=(k == 0), stop=(k == NK1 - 1))
                if f < 8:
                    nc.scalar.copy(gate_sb[:, f * SB:(f + 1) * SB], acc[:])
                elif f < 18:
                    t = f - 8
                    base = t * (SB + 3)
                    nc.scalar.copy(ccat[:, base + 3:base + 3 + SB], acc[:])
                else:
                    nc.scalar.copy(dtraw[:, :], acc[:HL, :])

            for f in [18] + list(range(18)):
                emit_ftile(f)
                for _ in range(2):
                    if pending_out:
                        emit_outproj(*pending_out.pop(0))

            nc.scalar.activation(gate_sb[:], gate_sb[:], AF.Silu)

            # ---------------- dt pipeline ----------------
            # softplus(z) = relu(z) + log1p(exp(-|z|)); log1p via poly
            # (no Softplus/Ln activation table on gen3)
            uu = dtp.tile([HL, SB], F32, tag="uu")
            pp = dtrp.tile([HL, SB], F32, tag="pp")
            dtsp = dtraw  # in-place: relu(z) overwrites z
            cs = pp       # reuse pp once the poly is folded in
            nc.scalar.activation(dtraw[:], dtraw[:], AF.Identity,
                                 bias=dtb_sb[:, 0:1])
            # uu = exp(min(z, -z)) = exp(-|z|)
            nc.vector.tensor_scalar(uu[:], dtraw[:], -1.0, None, ALU.mult)
            nc.vector.tensor_tensor(uu[:], uu[:], dtraw[:], ALU.min)
            nc.scalar.activation(uu[:], uu[:], AF.Exp)
            # Horner for q(u) = log1p(u)/u
            nc.vector.tensor_scalar(pp[:], uu[:], LOG1P_C[-1], LOG1P_C[-2],
                                    ALU.mult, ALU.add)
            for cidx in range(len(LOG1P_C) - 3, -1, -1):
                nc.vector.tensor_tensor(pp[:], pp[:], uu[:], ALU.mult)
                nc.vector.tensor_scalar(pp[:], pp[:], LOG1P_C[cidx], None,
                                        ALU.add)
            nc.vector.tensor_tensor(pp[:], pp[:], uu[:], ALU.mult)
            relu_t = uu  # uu dead; use as relu scratch
            nc.scalar.activation(relu_t[:], dtraw[:], AF.Relu)
            nc.vector.tensor_tensor(dtsp[:], relu_t[:], pp[:], ALU.add)
            nc.vector.tensor_scalar(dtsp[:], dtsp[:], DT_MIN, DT_MAX,
                                    ALU.max, ALU.min)
            dA = uu  # reuse again (relu scratch is dead)
            nc.vector.tensor_scalar(dA[:], dtsp[:], a_sb[:, 0:1], None,
                                    ALU.mult)
            for cl in range(NCPB):
                nc.vector.tensor_tensor_scan(
                    cs[:, cl * CHUNK:(cl + 1) * CHUNK],
                    ones16[:], dA[:, cl * CHUNK:(cl + 1) * CHUNK],
                    0.0, ALU.mult, ALU.add)

            # ---------------- SSD chunks ----------------
            qstage = gqp.tile([128, NK2 * SB], F32R, tag="gq")

            def emit_conv(cl):
                xc = cchp.tile([128, 8 * CHUNK], F32, tag="xc")
                bcs = ch1p.tile([128, CHUNK], F32, tag="bc")
                ccs = ch1p.tile([128, CHUNK], F32, tag="cc")
                for t in range(10):
                    base = t * (SB + 3) + cl * CHUNK
                    dst = (xc[:, t * CHUNK:(t + 1) * CHUNK] if t < 8
                           else (bcs[:] if t == 8 else ccs[:]))
                    nc.vector.tensor_scalar(
                        dst, ccat[:, base:base + CHUNK],
                        cw_sb[:, t * KCONV:t * KCONV + 1], cb_sb[:, t:t + 1],
                        ALU.mult, ALU.add)
                    for j in range(1, KCONV):
                        nc.vector.scalar_tensor_tensor(
                            dst, ccat[:, base + j:base + j + CHUNK],
                            cw_sb[:, t * KCONV + j:t * KCONV + j + 1], dst,
                            ALU.mult, ALU.add)
                    nc.scalar.activation(dst, dst, AF.Silu)
                return xc, bcs, ccs

            for cl in range(NCPB):
                ch = sb * NCPB + cl
                csl = slice(cl * CHUNK, (cl + 1) * CHUNK)
                xc, bcs, ccs = emit_conv(cl)

                # gate transpose + SiLU
                gps = psT.tile([128, CLOC], F32, tag="trans")
                for t in range(8):
                    nc.tensor.transpose(
                        gps[:, t * 128:(t + 1) * 128],
                        gate_sb[:, t * SB + cl * CHUNK:t * SB + (cl + 1) * CHUNK],
                        id_sb[:])
                silg = ch1p.tile([128, CLOC], F32, tag="silg")
                nc.scalar.copy(silg[:], gps[:])

                # small transposes: csT, dtT
                pT = psS.tile([128, 128], F32, tag="small")
                nc.tensor.transpose(pT[:, :HL], cs[:, csl], id_sb[:HL, :HL])
                csT = chp.tile([128, HL], F32, tag="csT")
                negcsT = chp.tile([128, HL], F32, tag="negcsT")
                nc.scalar.copy(csT[:], pT[:, :HL])
                nc.scalar.mul(negcsT[:], pT[:, :HL], -1.0)

                pT2 = psS.tile([128, 128], F32, tag="small")
                nc.tensor.transpose(pT2[:, :HL], dtsp[:, csl], id_sb[:HL, :HL])
                dtT = chp.tile([128, HL], F32, tag="dtT")
                nc.scalar.copy(dtT[:], pT2[:, :HL])

                # cs at chunk end, broadcast across partitions (PE matmul)
                pT3 = psS.tile([128, 128], F32, tag="small")
                e127b = bass.AP(tensor=e127_sb.tensor,
                                offset=e127_sb[:].offset,
                                ap=[[e127_sb[:].ap[0][0], 128], [0, 128]])
                nc.tensor.matmul(pT3[:, :HL], e127b, csT[:],
                                 start=True, stop=True)
                cdbc = chp.tile([128, HL], F32, tag="cdbc")
                decT = chp.tile([128, HL], F32, tag="decT")
                nc.scalar.activation(cdbc[:], pT3[:, :HL], AF.Exp)
                nc.vector.tensor_tensor(decT[:], pT3[:, :HL], csT[:],
                                        ALU.subtract)
                nc.scalar.activation(decT[:], decT[:], AF.Exp)
                ddt = chp.tile([128, HL], F32, tag="ddt")
                nc.vector.tensor_tensor(ddt[:], dtT[:], decT[:], ALU.mult)

                # x transpose -> xT, then xdt / xdd
                xps = psT.tile([128, CLOC], F32, tag="trans")
                for t in range(8):
                    nc.tensor.transpose(
                        xps[:, t * 128:(t + 1) * 128],
                        xc[:, t * CHUNK:(t + 1) * CHUNK], id_sb[:])
                xT = ch1p.tile([128, CLOC], F32, tag="xT")
                nc.scalar.copy(xT[:], xps[:])
                xdt = ch1p.tile([128, CLOC], F32, tag="xdt")
                xdd = ch1p.tile([128, CLOC], F32R, tag="xdd")
                for h in range(HL):
                    hs = slice(h * HD, (h + 1) * HD)
                    nc.vector.tensor_scalar(
                        xdt[:, hs], xT[:, hs], dtT[:, h:h + 1], None, ALU.mult)
                for h in range(HL):
                    hs = slice(h * HD, (h + 1) * HD)
                    nc.vector.tensor_scalar(
                        xdd[:, hs], xT[:, hs], ddt[:, h:h + 1], None, ALU.mult)

                # B chunk transposed (B_LN)
                pbt = psS.tile([128, 128], F32, tag="small")
                nc.tensor.transpose(pbt[:], bcs[:], id_sb[:])
                bln = chp.tile([128, 128], F32R, tag="bln")
                nc.scalar.copy(bln[:], pbt[:])

                # Gram^T = B C^T in [s, l]; evicted to SBUF
                gram_ps = psS.tile([128, 128], F32, tag="small")
                nc.tensor.matmul(gram_ps[:], bcs[:], ccs[:],
                                 start=True, stop=True)
                gram = ch1p.tile([128, 128], F32, tag="gram")
                nc.scalar.copy(gram[:], gram_ps[:])

                # per-head decay matrices in groups of 4 heads
                y_ps = psY.tile([128, CLOC], F32, tag="yo")
                for g in range(HL // 4):
                    pb4 = psS.tile([128, 512], F32, tag="small")
                    for j in range(4):
                        h = 4 * g + j
                        idcol = id_sb[:HL, h:h + 1]
                        indh = bass.AP(tensor=idcol.tensor,
                                       offset=idcol.offset,
                                       ap=[[idcol.ap[0][0], HL], [0, 128]])
                        nc.tensor.matmul(pb4[:, j * 128:(j + 1) * 128], indh,
                                         cs[:, csl], start=True, stop=True)
                    epb4 = hp.tile([128, 512], F32, tag="epb")
                    nc.scalar.activation(epb4[:], pb4[:], AF.Exp)
                    seg4 = hp.tile([128, 512], F32, tag="seg")
                    for j in range(4):
                        h = 4 * g + j
                        nc.vector.scalar_tensor_tensor(
                            seg4[:, j * 128:(j + 1) * 128],
                            pb4[:, j * 128:(j + 1) * 128],
                            negcsT[:, h:h + 1], nm_sb[:], ALU.add, ALU.add)
                    nc.scalar.activation(seg4[:], seg4[:], AF.Exp)
                    gram_b = bass.AP(tensor=gram.tensor, offset=gram[:].offset,
                                     ap=[gram[:].ap[0], [0, 4], [1, 128]])
                    ccs_b = bass.AP(tensor=ccs.tensor, offset=ccs[:].offset,
                                    ap=[ccs[:].ap[0], [0, 4], [1, 128]])
                    s4 = seg4[:].rearrange("p (j l) -> p j l", j=4)
                    e4 = epb4[:].rearrange("p (j l) -> p j l", j=4)
                    nc.vector.tensor_tensor(s4, s4, gram_b, ALU.mult)
                    nc.vector.tensor_tensor(e4, e4, ccs_b, ALU.mult)
                    for j in range(4):
                        h = 4 * g + j
                        hs = slice(h * HD, (h + 1) * HD)
                        nc.tensor.matmul(
                            y_ps[:, hs], seg4[:, j * 128:(j + 1) * 128],
                            xdt[:, hs], start=True, stop=False)
                        nc.tensor.matmul(
                            y_ps[:, hs], epb4[:, j * 128:(j + 1) * 128],
                            st_sb[:, hs], start=False, stop=True)

                # states for this chunk
                s_ps = psY.tile([128, CLOC], F32, tag="yo")
                for half in range(2):
                    hsl = slice(half * 512, (half + 1) * 512)
                    nc.tensor.matmul(
                        s_ps[:, hsl], bln[:], xdd[:, hsl],
                        start=True, stop=True)

                # y = (Ydiag + Yoff) + D*x ; state update
                y_sb = ch1p.tile([128, CLOC], F32, tag="ysb")
                for h in range(HL):
                    hs = slice(h * HD, (h + 1) * HD)
                    nc.vector.scalar_tensor_tensor(
                        y_sb[:, hs], xT[:, hs], dbc_sb[:, h:h + 1],
                        y_ps[:, hs], ALU.mult, ALU.add)
                for h in range(HL):
                    hs = slice(h * HD, (h + 1) * HD)
                    nc.vector.scalar_tensor_tensor(
                        st_sb[:, hs], st_sb[:, hs], cdbc[:, h:h + 1],
                        s_ps[:, hs], ALU.mult, ALU.add)

                # gate + group RMSNorm
                nc.vector.tensor_tensor(y_sb[:], y_sb[:], silg[:], ALU.mult)
                ssum = ch1p.tile([128, 1], F32, tag="ssum")
                # Square's main output is discarded into xdd (scratch)
                nc.scalar.activation(xdd[:], y_sb[:], AF.Square,
                                     accum_out=ssum[:, 0:1])
                nc.vector.tensor_scalar(ssum[:], ssum[:], 1.0 / GROUP, EPS,
                                        ALU.mult, ALU.add)
                rstd = chp.tile([128, 1], F32, tag="rstd")
                tnew = chp.tile([128, 1], F32, tag="tnew")
                nc.scalar.activation(tnew[:], ssum[:], AF.Sqrt)
                nc.vector.reciprocal(rstd[:], tnew[:])
                normed = ch1p.tile([128, CLOC], F32, tag="normed")
                nc.vector.tensor_scalar(
                    normed[:], y_sb[:], rstd[:, 0:1], None, ALU.mult)

                # transpose normed -> [c, s] and stage out to DRAM
                nps = psT.tile([128, CLOC], F32, tag="trans")
                for t in range(8):
                    nc.tensor.transpose(
                        nps[:, t * 128:(t + 1) * 128],
                        normed[:, t * 128:(t + 1) * 128], id_sb[:])
                qdst = qstage[:].rearrange(
                    "p (t s) -> p t s", t=NK2)[:, :, cl * 128:(cl + 1) * 128]
                nsrc = nps[:].rearrange("p (t s) -> p t s", t=NK2)
                nc.scalar.copy(qdst, nsrc)

            # out_proj m-blocks are deferred and interleaved into the
            # next superblock's in_proj f-loop (shared psA rotation)
            pending_out.extend((m, qstage, sb) for m in range(NM2))

        while pending_out:
            emit_outproj(*pending_out.pop(0))


def prepare_in_maps(hidden_states, in_proj_w, conv_w, conv_b, dt_bias, D,
                    norm_w, out_proj_w):
    hidT = np.ascontiguousarray(hidden_states.reshape(S, H_SIZE).T)
    # [half, kk, r, sb, c] -> [sb, half, r, kk, c]
    hids = np.ascontiguousarray(
        hidT.reshape(2, 16, 128, NSB, SB).transpose(3, 0, 2, 1, 4)
        .reshape(NSB, 2, 128, 16 * SB))
    negmask = np.where(np.arange(128)[None, :] >= np.arange(128)[:, None],
                       np.float32(0.0), np.float32(NEGM)).astype(np.float32)
    ident = np.eye(128, dtype=np.float32)
    e127 = np.zeros((128, 1), np.float32)
    e127[127, 0] = 1.0
    in_maps = []
    for c in range(N_CORES):
        gsl = slice(CLOC * c, CLOC * (c + 1))
        xsl = slice(INTER + CLOC * c, INTER + CLOC * (c + 1))
        bsl = slice(2 * INTER + SS * c, 2 * INTER + SS * (c + 1))
        cslc = slice(2 * INTER + NG * SS + SS * c,
                     2 * INTER + NG * SS + SS * (c + 1))
        dsl = slice(INTER + CONV_DIM + HL * c, INTER + CONV_DIM + HL * (c + 1))
        w1 = np.concatenate([in_proj_w[gsl], in_proj_w[xsl], in_proj_w[bsl],
                             in_proj_w[cslc], in_proj_w[dsl]], axis=0)
        w1 = np.concatenate(
            [w1, np.zeros((NF * 128 - w1.shape[0], H_SIZE), np.float32)],
            axis=0)
        # W1T [4096, 2432]: [half, kk, r, f, fc] -> [f, half, r, kk, fc]
        w1f = np.ascontiguousarray(
            w1.T.reshape(2, 16, 128, NF, 128).transpose(3, 0, 2, 1, 4)
            .reshape(NF, 2, 128, 16 * 128))
        w2 = out_proj_w[:, gsl] * norm_w[gsl][None, :]  # norm_w folded
        # W2T [1024, 4096]: [kt, r, m, mc] -> [m, r, kt, mc]
        w2m = np.ascontiguousarray(
            w2.T.reshape(NK2, 128, NM2, 128).transpose(2, 1, 0, 3)
            .reshape(NM2, 128, NK2 * 128))
        conv_idx = np.concatenate([
            np.arange(CLOC * c, CLOC * (c + 1)),
            np.arange(INTER + SS * c, INTER + SS * (c + 1)),
            np.arange(INTER + NG * SS + SS * c,
                      INTER + NG * SS + SS * (c + 1))])
        cwl = conv_w[conv_idx, 0, :]          # [1280, 4]
        cbl = conv_b[conv_idx]                # [1280]
        convw = np.ascontiguousarray(
            cwl.reshape(10, 128, KCONV).transpose(1, 0, 2)
            .reshape(128, 10 * KCONV))
        convb = np.ascontiguousarray(cbl.reshape(10, 128).transpose(1, 0))
        hsl = slice(HL * c, HL * (c + 1))
        acol = -(np.arange(HL * c + 1, HL * (c + 1) + 1, dtype=np.float32))
        in_maps.append({
            "hids": hids,
            "w1f": w1f,
            "w2m": w2m,
            "convw": convw,
            "convb": convb,
            "dtbias": dt_bias[hsl].reshape(HL, 1).astype(np.float32),
            "acol": acol.reshape(HL, 1),
            "dbc": np.tile(D[hsl][None, :], (128, 1)).astype(np.float32),
            "negmask": negmask,
            "ident": ident,
            "e127": e127,
        })
    return in_maps


def get_nc():
    if "nc" not in _CACHE:
        _CACHE["nc"] = build_nc()
    return _CACHE["nc"]


def kernel(hidden_states, in_proj_w, conv_w, conv_b, dt_bias, D, norm_w,
           out_proj_w):
    nc = get_nc()
    in_maps = prepare_in_maps(
        np.asarray(hidden_states, np.float32),
        np.asarray(in_proj_w, np.float32),
        np.asarray(conv_w, np.float32), np.asarray(conv_b, np.float32),
        np.asarray(dt_bias, np.float32), np.asarray(D, np.float32),
        np.asarray(norm_w, np.float32), np.asarray(out_proj_w, np.float32))
    res = run_bass_kernel_spmd(nc, in_maps, list(range(N_CORES)))
    acc = np.zeros((H_SIZE, S), np.float64)
    for r in res.results:
        acc += r["outp"].transpose(0, 2, 1, 3).reshape(H_SIZE, S)
    return acc.T.astype(np.float32).reshape(1, S, H_SIZE)
